# revision 12
# baseline (speedup 1.0000x reference)
"""Trainium2 Bass kernel: multi-head self-attention block (B=16, N=1024, C=768, H=12).

Data-parallel over batch: 8 NeuronCores x 2 batches each, no collectives.

v4 (from v2 baseline ~325us; PE streaming floor ~290us):
  * Host prepares CONTIGUOUS dram layouts per SBUF tile (wq/wk/wv split out
    of W_qkv, x chunk-major per batch): every prologue DMA is a full-rate
    contiguous read instead of a strided slab at ~1/3 bandwidth.
  * Ramp-critical stream split across both queues (sync: wq-hp0 chunks + x0
    even chunks; gpsimd: wk-hp0 + x0 odd chunks), cc-major, so the first qk
    matmul fires early and chases the stream. Bulk follows in-queue.
  * Norm: both heads' U-psum evacuations happen before the z/recip/mul math
    (frees the flex PSUM ring early).
  * Epilogue: ALL 16 proj(b1) groups pre-accumulate cc0..4 into a rotating
    PSUM ring and evacuate partials into the y tiles (Scalar engine does the
    copies - it is idle once the last exp is done) while the final norm chain
    drains. After it, only 16 single-matmul cc5 "finals" + in-place adds +
    wide y DMAs on both queues remain.
  * b_proj applied on host (it is a free elementwise add on the output).

Dataflow per core (all-transposed activations; no on-chip transposes):
  host: xT = x_shard^T                                  [C, T]
  qT/kT(hp,b) = Wq/Wk^T-slices @ xT(b)                  [128, N]
  v'   = xT-tiles^T @ Wv  (+ ones col/head)             [N, H*(HD+1)]
  S^T  = k^T-slices^T @ q^T   (per head, K=64)          [Nk, Nq]
  E    = exp(SCALE * S^T)     (ScalarE, PSUM->SBUF)     [Nk, Nq]
  U'   = v'^T @ E  (accum over k; row HD = softmax Z)   [HD+1, Nq]
  aoT  = U'[:HD] * (1/Z broadcast)                      [C, N]
  y    = aoT-tiles^T @ W_proj                           [N, C]
"""

import sys

for _p in ("/opt/trn_rl_repo", "/opt/pypackages"):
    if _p not in sys.path:
        sys.path.append(_p)

import numpy as np

B, N, C, H = 16, 1024, 768, 12
HD = C // H            # 64
SCALE = HD ** -0.5
NCORES = 8
BL = B // NCORES       # 2 batches per core
T = BL * N             # 2048 tokens per core

COMPUTE = "bf16"       # "bf16" | "f32r"


def build_attention_nc(compute=COMPUTE, bl=BL, n=N, c=C, h=H):
    import concourse.bass as bass
    import concourse.tile as tile
    from concourse import bacc, mybir
    from contextlib import ExitStack

    hd = c // h
    t = bl * n
    scale = hd ** -0.5
    assert c % 128 == 0 and n % 512 == 0 and h % 2 == 0 and hd == 64
    CCH = c // 128      # contraction chunks over channels (6)
    NHP = h // 2        # head pairs (6)
    NQ = n // 512       # 512-wide q tiles per sequence (2)
    NKT = n // 128      # 128-wide k tiles per sequence (8)
    NTT = n // 128      # 128-wide token tiles per sequence (8)
    VW = hd + 1         # v' width per head (ones col at hd)
    PH = c // 2         # proj/v free-dim half (384), <= 1 PSUM bank
    NXH = n // 512      # 512-col x halves per batch (2)

    FP32 = mybir.dt.float32
    SD = mybir.dt.bfloat16 if compute == "bf16" else FP32  # storage dtype

    def mm(ap):
        return ap.bitcast(mybir.dt.float32r) if compute == "f32r" else ap

    nc = bacc.Bacc("TRN2", target_bir_lowering=False, debug=False,
                   num_devices=NCORES)

    # host-side contiguous layouts (see make_in_maps):
    #   xT_d rows: (b, cc, p) -> [bl*c, n];  w*_d: [c, c] column-split W_qkv
    xT_d = nc.dram_tensor("xT", [bl * c, n], SD, kind="ExternalInput").ap()
    wq_d = nc.dram_tensor("w_q", [c, c], SD, kind="ExternalInput").ap()
    wk_d = nc.dram_tensor("w_k", [c, c], SD, kind="ExternalInput").ap()
    wv_d = nc.dram_tensor("w_v", [c, c], SD, kind="ExternalInput").ap()
    # hp0 slices of wq+wk stacked on host: one fast ramp DMA
    wqk0_d = nc.dram_tensor("w_qk0", [2 * c, 128], SD, kind="ExternalInput").ap()
    wproj_d = nc.dram_tensor("w_proj", [c, c], SD, kind="ExternalInput").ap()
    out_d = nc.dram_tensor("out", [t, c], FP32, kind="ExternalOutput").ap()

    Exp = mybir.ActivationFunctionType.Exp
    Copy = mybir.ActivationFunctionType.Copy

    units = [(b, hp) for b in range(bl) for hp in range(NHP)]
    NU = len(units)     # 12

    with tile.TileContext(nc) as tc, ExitStack() as ctx:
        consts = ctx.enter_context(tc.tile_pool(name="consts", bufs=1))
        xp = ctx.enter_context(tc.tile_pool(name="xp", bufs=2))
        qkp = ctx.enter_context(tc.tile_pool(name="qkp", bufs=2))
        vp = ctx.enter_context(tc.tile_pool(name="vp", bufs=2))
        ep = ctx.enter_context(tc.tile_pool(name="ep", bufs=9))
        aop = ctx.enter_context(tc.tile_pool(name="aop", bufs=2))
        smp = ctx.enter_context(tc.tile_pool(name="smp", bufs=1))
        yp = ctx.enter_context(tc.tile_pool(name="yp", bufs=8))
        ps_s = ctx.enter_context(tc.tile_pool(name="ps_s", bufs=2, space="PSUM"))
        ps_f = ctx.enter_context(tc.tile_pool(name="ps_f", bufs=4, space="PSUM"))

        # ---------------- DMA prologue --------------------------------------
        # Few big merged DMAs (issue costs ~610ns of queue time each) on
        # FOUR queues: sync carries the ramp-critical stream (hp0 weights +
        # x(b0) in three 2-chunk pieces, then wq), gpsimd wv-then-wk,
        # vector x(b1), scalar wproj. All sources contiguous in dram.
        wqk0_sb = consts.tile([128, 2 * CCH, 128], SD, tag="wqk0")
        wq_hp0 = [wqk0_sb[:, cc, :] for cc in range(CCH)]
        wk_hp0 = [wqk0_sb[:, CCH + cc, :] for cc in range(CCH)]
        NXG = CCH // 2      # x chunk-pair groups (3)
        xg = [[xp.tile([128, 2, n], SD, tag=f"xg{g}", name=f"xg_b{b}g{g}")
               for g in range(NXG)] for b in range(bl)]
        xT_all = [[[xg[b][cc // 2][:, cc % 2, xh * 512:(xh + 1) * 512]
                    for xh in range(NXH)] for cc in range(CCH)]
                  for b in range(bl)]

        def rr(dram_ap):
            return dram_ap.rearrange("(j p) f -> p j f", p=128)

        nc.sync.dma_start(out=wqk0_sb, in_=rr(wqk0_d[:, :]))
        for g in range(NXG):
            nc.sync.dma_start(out=xg[0][g],
                              in_=rr(xT_d[g * 256:(g + 1) * 256, :]))
        wqt = consts.tile([128, CCH, c], SD, tag="wqt")
        wkt = consts.tile([128, CCH, c], SD, tag="wkt")
        wvt = consts.tile([128, CCH, c], SD, tag="wvt")
        wpt = consts.tile([128, CCH, c], SD, tag="wpt")
        wq_sb = [wqt[:, cc, :] for cc in range(CCH)]
        wk_sb = [wkt[:, cc, :] for cc in range(CCH)]
        wv_sb = [wvt[:, cc, :] for cc in range(CCH)]
        wproj_sb = [wpt[:, cc, :] for cc in range(CCH)]
        for hh in range(2):
            r0, r1 = hh * 3 * 128, (hh + 1) * 3 * 128
            nc.gpsimd.dma_start(out=wvt[:, hh * 3:(hh + 1) * 3, :],
                                in_=rr(wv_d[r0:r1, :]))
        for hh in range(2):
            r0, r1 = hh * 3 * 128, (hh + 1) * 3 * 128
            nc.sync.dma_start(out=wqt[:, hh * 3:(hh + 1) * 3, :],
                              in_=rr(wq_d[r0:r1, :]))
            nc.gpsimd.dma_start(out=wkt[:, hh * 3:(hh + 1) * 3, :],
                                in_=rr(wk_d[r0:r1, :]))
            nc.scalar.dma_start(out=wpt[:, hh * 3:(hh + 1) * 3, :],
                                in_=rr(wproj_d[r0:r1, :]))
        for g in range(NXG):
            q = nc.scalar if g < 2 else nc.gpsimd
            q.dma_start(out=xg[1][g],
                        in_=rr(xT_d[c + g * 256:c + (g + 1) * 256, :]))

        # ---------------- building-block emitters --------------------------
        qt_all = {}   # (b, hp) -> [128, n] q^T tile (2 heads stacked)
        kt_all = {}
        v_all = [[None] * NTT for _ in range(bl)]
        e_all = {}    # (b, hp, kt, head) -> E tile
        u_ps = {}     # (b, hp) -> [head][qn] psum accumulators
        ao_all = {}   # (b, hp) -> [128, n] normalized attention output^T
        y_tiles = {}  # (b, tt) -> ([128, c] tile, halves-finished count)

        def emit_qk_group(b, hp, dst, qn):
            """Project one 512-token slice of q^T (dst=0) or k^T (dst=1)."""
            key = (b, hp)
            store = qt_all if dst == 0 else kt_all
            if key not in store:
                store[key] = qkp.tile([128, n], SD, tag=f"qk{dst}",
                                      name=f"{'qk'[dst]}t_b{b}hp{hp}")
            ps = ps_f.tile([128, 512], FP32, tag="u",
                           name=f"qkps_b{b}hp{hp}d{dst}q{qn}")
            for cc in range(CCH):
                if b == 0 and hp == 0:
                    w_ap = (wq_hp0 if dst == 0 else wk_hp0)[cc]
                else:
                    w_sb = wq_sb if dst == 0 else wk_sb
                    w_ap = w_sb[cc][:, hp * 128:(hp + 1) * 128]
                nc.tensor.matmul(
                    ps,
                    lhsT=mm(w_ap),
                    rhs=mm(xT_all[b][cc][qn]),
                    start=(cc == 0), stop=(cc == CCH - 1))
            with tc.high_priority(offset=300):
                nc.vector.tensor_copy(
                    store[key][:, qn * 512:(qn + 1) * 512], ps)

        def emit_v_group(b, tt, half):
            """One [128-token, 384-channel] slice of v' (+ones cols)."""
            if half == 0:
                vt = vp.tile([128, h * VW], SD, tag=f"v{tt}",
                             name=f"v_b{b}t{tt}")
                ones_view = vt[:, :].rearrange(
                    "p (hh w) -> p hh w", hh=h)[:, :, hd:hd + 1]
                nc.gpsimd.memset(ones_view, 1.0)
                v_all[b][tt] = vt
            vt = v_all[b][tt]
            ps = ps_f.tile([128, PH], FP32, tag="u",
                           name=f"vps_b{b}t{tt}f{half}")
            xh, tl = tt // 4, tt % 4
            for cc in range(CCH):
                nc.tensor.matmul(
                    ps,
                    lhsT=mm(xT_all[b][cc][xh][:, tl * 128:(tl + 1) * 128]),
                    rhs=mm(wv_sb[cc][:, half * PH:(half + 1) * PH]),
                    start=(cc == 0), stop=(cc == CCH - 1))
            nheads = PH // hd
            dst = vt[:, half * nheads * VW:(half + 1) * nheads * VW].rearrange(
                "p (hh w) -> p hh w", hh=nheads)[:, :, 0:hd]
            srcv = ps[:].rearrange("p (hh w) -> p hh w", hh=nheads)
            with tc.high_priority(offset=300):
                nc.vector.tensor_copy(dst, srcv)

        def get_y_tile(b, tt):
            if (b, tt) not in y_tiles:
                y_tiles[(b, tt)] = [yp.tile([128, c], FP32, tag="y",
                                            name=f"y_b{b}t{tt}"), 0]
            return y_tiles[(b, tt)]

        tail_q = [None]

        def evac_y_half(b, tt, half, ps, evac, bump=True, dma_half=False):
            """Copy/accumulate proj psum into the y tile; DMA when complete."""
            ent = get_y_tile(b, tt)
            yt = ent[0]
            dstv = yt[:, half * PH:(half + 1) * PH]
            if evac == "scalar":
                nc.scalar.activation(dstv, ps, Copy)
            elif evac == "add":
                with tc.high_priority(offset=300):
                    nc.vector.tensor_add(dstv, ps, dstv)
            else:
                with tc.high_priority(offset=300):
                    nc.vector.tensor_copy(dstv, ps)
            if dma_half:
                # tail mode: ship each half as soon as it is final,
                # round-robin over three DMA queues
                qs = [nc.sync, nc.gpsimd, nc.scalar]
                tail_q[0] = (tail_q[0] + 1) if tail_q[0] is not None else 0
                qs[tail_q[0] % 3].dma_start(
                    out=out_d[b * n + tt * 128:b * n + (tt + 1) * 128,
                              half * PH:(half + 1) * PH],
                    in_=dstv)
                return
            if bump:
                ent[1] += 1
                if ent[1] == 2:
                    q = nc.sync if (b == 0 or tt % 2 == 0) else nc.gpsimd
                    q.dma_start(
                        out=out_d[b * n + tt * 128:b * n + (tt + 1) * 128, :],
                        in_=yt)
                    del y_tiles[(b, tt)]

        def emit_proj_group(b, tt, half, evac="vector"):
            """One [128-token, 384-channel] output-projection slice."""
            ps = ps_f.tile([128, PH], FP32, tag="u",
                           name=f"yps_b{b}t{tt}f{half}")
            for cc in range(CCH):
                nc.tensor.matmul(
                    ps,
                    lhsT=mm(ao_all[(b, cc)][:, tt * 128:(tt + 1) * 128]),
                    rhs=mm(wproj_sb[cc][:, half * PH:(half + 1) * PH]),
                    start=(cc == 0), stop=(cc == CCH - 1))
            evac_y_half(b, tt, half, ps, evac)

        def emit_S(b, hp, kt):
            """S^T matmuls + exp for both heads of one 128-key tile."""
            qb = qt_all[(b, hp)]
            kb = kt_all[(b, hp)]
            for head in range(2):
                p0 = head * 64
                sps = ps_s.tile([128, n], FP32, tag="s",
                                name=f"s_b{b}hp{hp}k{kt}h{head}")
                for qn in range(NQ):
                    nc.tensor.matmul(
                        sps[:, qn * 512:(qn + 1) * 512],
                        lhsT=mm(kb[p0:p0 + 64, kt * 128:(kt + 1) * 128]),
                        rhs=mm(qb[p0:p0 + 64, qn * 512:(qn + 1) * 512]),
                        start=True, stop=True)
                et = ep.tile([128, n], SD, tag=f"e{head}",
                             name=f"e_b{b}hp{hp}k{kt}h{head}")
                nc.scalar.activation(et, sps, Exp, scale=scale)
                e_all[(b, hp, kt, head)] = et

        def emit_U_chunk(b, hp, kts):
            """U matmuls for key-tiles `kts`; their E tiles are all ready."""
            if (b, hp) not in u_ps:
                u_ps[(b, hp)] = [[ps_f.tile([VW, 512], FP32, tag="u",
                                            name=f"u_b{b}hp{hp}h{hh}q{qn}")
                                  for qn in range(NQ)] for hh in range(2)]
            ups = u_ps[(b, hp)]
            for kt in kts:
                for head in range(2):
                    hh = 2 * hp + head
                    et = e_all.pop((b, hp, kt, head))
                    for qn in range(NQ):
                        nc.tensor.matmul(
                            ups[head][qn],
                            lhsT=mm(v_all[b][kt][:, hh * VW:hh * VW + VW]),
                            rhs=mm(et[:, qn * 512:(qn + 1) * 512]),
                            start=(kt == 0), stop=(kt == NKT - 1))

        def emit_norm_evac(b, hp):
            """Copy both heads' U psums to SBUF (frees the flex ring)."""
            ups = u_ps.pop((b, hp))
            usb = {}
            for head in (1, 0):
                ut = smp.tile([VW, n], FP32, tag=f"usb{head}",
                              name=f"usb_b{b}hp{hp}h{head}")
                for qn in range(NQ):
                    with tc.high_priority(offset=300):
                        nc.vector.tensor_copy(
                            ut[:, qn * 512:(qn + 1) * 512], ups[head][qn])
                usb[head] = ut
            return usb

        def emit_norm_math(b, hp, usb):
            """Divide by Z, build aoT tile for proj.

            Z -> partition 0 (DMA), broadcast to 64 partitions (gpsimd),
            reciprocal on the full tile (base-partition!=0 sources
            mis-execute on hw for both the DVE op and the broadcast).
            """
            ao = aop.tile([128, n], SD, tag=f"ao{hp}", name=f"ao_b{b}hp{hp}")
            ao_all[(b, hp)] = ao
            for head in (1, 0):
                ut = usb[head]
                z1 = smp.tile([1, n], FP32, tag=f"z1{head}", bufs=1,
                              name=f"z1_b{b}hp{hp}h{head}")
                nc.sync.dma_start(out=z1, in_=ut[hd:hd + 1, :])
                rb = smp.tile([64, n], FP32, tag=f"rb{head}",
                              name=f"rb_b{b}hp{hp}h{head}")
                nc.gpsimd.partition_broadcast(rb, z1)
                nc.vector.reciprocal_approx_fast(rb, rb)
                if head == 0:
                    nc.vector.tensor_mul(ao[0:64, :], ut[0:hd, :], rb)
                else:
                    sc = smp.tile([64, n], SD, tag="sc",
                                  name=f"sc_b{b}hp{hp}")
                    nc.vector.tensor_mul(sc, ut[0:hd, :], rb)
                    nc.sync.dma_start(out=ao[64:128, :], in_=sc)

        def emit_norm(b, hp):
            emit_norm_math(b, hp, emit_norm_evac(b, hp))

        # ---------------- filler schedule ----------------------------------
        # per-unit list of thunks run between S groups of that unit
        fillers = [[] for _ in range(NU)]

        def add_qk_fillers(i, b, hp):
            for qn in range(NQ):
                for dst in range(2):
                    fillers[i].append(
                        lambda b=b, hp=hp, dst=dst, qn=qn:
                        emit_qk_group(b, hp, dst, qn))

        # unit 0: v(b0) fully + qk(unit1)
        for tt in range(NTT):
            for half in range(2):
                fillers[0].append(
                    lambda tt=tt, half=half: emit_v_group(0, tt, half))
        add_qk_fillers(0, *units[1])
        # units 1..4: qk(next) + v(b1) spread 4 per unit
        for i in range(1, 5):
            add_qk_fillers(i, *units[i + 1])
        vq = [(tt, half) for tt in range(NTT) for half in range(2)]
        for j, (tt, half) in enumerate(vq):
            fillers[1 + j // 4].append(
                lambda tt=tt, half=half: emit_v_group(1, tt, half))
        # units 5..10: qk(next)
        for i in range(5, 11):
            add_qk_fillers(i, *units[i + 1])
        # units 7..10: proj(b0)  (all ao(b0) ready after norm(u5) in unit 6;
        # unit 11 keeps its flex psum free for in-unit U accumulation)
        pq = [(tt, half) for tt in range(NTT) for half in range(2)]
        for j, (tt, half) in enumerate(pq):
            fillers[7 + j % 4].append(
                lambda tt=tt, half=half: emit_proj_group(0, tt, half))

        # ---------------- main schedule ------------------------------------
        # prologue: qk(unit0), qn-major to chase the x DMA stream
        b0, hp0 = units[0]
        for qn in range(NQ):
            for dst in range(2):
                emit_qk_group(b0, hp0, dst, qn)

        for i, (b, hp) in enumerate(units):
            fl = list(fillers[i])
            prev = units[i - 1] if i > 0 else None
            # S(k0) first so the Scalar engine stays fed across the boundary;
            # previous unit's U matmuls run in chunks between S groups so
            # exp never starves and every U operand is long since ready.
            emit_S(b, hp, 0)
            start_kt = 1
            if prev is not None:
                for j, (k0, k1) in enumerate(((0, 2), (2, 4), (4, 6), (6, 8))):
                    emit_U_chunk(prev[0], prev[1], range(k0, k1))
                    if j < 3:
                        emit_S(b, hp, j + 1)
                emit_norm(*prev)
                start_kt = 4
            # spread fillers across the remaining kt slots; the last unit
            # instead runs its own U matmuls in-unit at lag 4
            nslots = NKT - start_kt
            tot = len(fl)
            for kt in range(start_kt, NKT):
                emit_S(b, hp, kt)
                if i == NU - 1 and kt >= 4:
                    emit_U_chunk(b, hp, [kt - 4])
                j = kt - start_kt
                k = (tot * (j + 1)) // nslots - (tot * j) // nslots
                for _ in range(k):
                    if fl:
                        fl.pop(0)()

        # ---------------- epilogue -----------------------------------------
        # Last unit's U(k4..7); then cc0..4 partials for ALL 16 proj(b1)
        # groups on a rotating PSUM ring, evacuated into y tiles while the
        # final norm chain drains; then 16 single-matmul cc5 finals with
        # in-place adds and wide y DMAs on both queues.
        pb, php = units[-1]
        emit_U_chunk(pb, php, range(4, NKT))

        groups = [(tt, half) for tt in range(NTT) for half in range(2)]
        s_carve = [ps_s.tile([128, n], FP32, tag="s", name=f"scarve{j}")
                   for j in range(2)]

        def partial_ps(g):
            if g % 2 == 0:
                base = s_carve[(g // 2) % 2]
                return base[:, (g // 4 % 2) * 512:(g // 4 % 2) * 512 + PH]
            return ps_f.tile([128, PH], FP32, tag="u", name=f"ypart{g}")

        def emit_partial(g, evac):
            tt, half = groups[g]
            ps = partial_ps(g)
            for cc in range(CCH - 1):
                nc.tensor.matmul(
                    ps,
                    lhsT=mm(ao_all[(1, cc)][:, tt * 128:(tt + 1) * 128]),
                    rhs=mm(wproj_sb[cc][:, half * PH:(half + 1) * PH]),
                    start=(cc == 0), stop=(cc == CCH - 2))
            evac_y_half(1, tt, half, ps, evac, bump=False)

        # first 4 partials ride on the freed S slots while U-psum evacuation
        # happens; the rest rotate the flex ring after it frees
        for g in range(4):
            emit_partial(g, "scalar")
        usb_last = emit_norm_evac(pb, php)
        for g in range(4, 16):
            emit_partial(g, "scalar" if g % 2 == 0 else "vector")
        emit_norm_math(pb, php, usb_last)
        # finals: cc5 matmul + in-place add, each half shipped immediately
        cc = CCH - 1
        for g, (tt, half) in enumerate(groups):
            ps = ps_f.tile([128, PH], FP32, tag="u", name=f"yfin{g}")
            nc.tensor.matmul(
                ps,
                lhsT=mm(ao_all[(1, cc)][:, tt * 128:(tt + 1) * 128]),
                rhs=mm(wproj_sb[cc][:, half * PH:(half + 1) * PH]),
                start=True, stop=True)
            evac_y_half(1, tt, half, ps, "add", dma_half=True)

    nc.compile()
    return nc


_NC_CACHE = {}


def _get_nc(compute=COMPUTE):
    if compute not in _NC_CACHE:
        _NC_CACHE[compute] = build_attention_nc(compute)
    return _NC_CACHE[compute]


def make_in_maps(x, W_qkv, W_proj, b_proj, compute=None):
    compute = compute or COMPUTE
    if compute == "bf16":
        import ml_dtypes
        sd = ml_dtypes.bfloat16
    else:
        sd = np.float32
    x = np.asarray(x, dtype=np.float32)
    W_qkv = np.asarray(W_qkv, dtype=np.float32)
    wq = np.ascontiguousarray(W_qkv[:, 0:C]).astype(sd)
    wk = np.ascontiguousarray(W_qkv[:, C:2 * C]).astype(sd)
    wv = np.ascontiguousarray(W_qkv[:, 2 * C:3 * C]).astype(sd)
    wqk0 = np.ascontiguousarray(np.concatenate(
        [W_qkv[:, 0:128], W_qkv[:, C:C + 128]], axis=0)).astype(sd)
    W_proj = np.ascontiguousarray(np.asarray(W_proj, dtype=np.float32)).astype(sd)
    in_maps = []
    for i in range(NCORES):
        shard = x[i * BL:(i + 1) * BL]                      # [BL, N, C]
        # rows ordered (b, c): contiguous [128, N] chunk reads
        xT = np.ascontiguousarray(
            shard.transpose(0, 2, 1).reshape(BL * C, N)).astype(sd)
        in_maps.append({"xT": xT, "w_q": wq, "w_k": wk, "w_v": wv,
                        "w_qk0": wqk0, "w_proj": W_proj})
    return in_maps


def kernel(x, W_qkv, W_proj, b_proj):
    from concourse.bass_utils import run_bass_kernel_spmd

    nc = _get_nc()
    in_maps = make_in_maps(x, W_qkv, W_proj, b_proj)
    res = run_bass_kernel_spmd(nc, in_maps, core_ids=list(range(NCORES)))
    outs = [res.results[i]["out"].reshape(BL, N, C) for i in range(NCORES)]
    y = np.concatenate(outs, axis=0).astype(np.float32)
    return y + np.asarray(b_proj, dtype=np.float32)


if __name__ == "__main__":
    nc = build_attention_nc()
    print("built ok")


# revision 21
# speedup vs baseline: 1.0366x; 1.0366x over previous
"""Trainium2 Bass kernel: multi-head self-attention block (B=16, N=1024, C=768, H=12).

Data-parallel over batch: 8 NeuronCores x 2 batches each, no collectives.

v4 (from v2 baseline ~325us; PE streaming floor ~290us):
  * Host prepares CONTIGUOUS dram layouts per SBUF tile (wq/wk/wv split out
    of W_qkv, x chunk-major per batch): every prologue DMA is a full-rate
    contiguous read instead of a strided slab at ~1/3 bandwidth.
  * Ramp-critical stream split across both queues (sync: wq-hp0 chunks + x0
    even chunks; gpsimd: wk-hp0 + x0 odd chunks), cc-major, so the first qk
    matmul fires early and chases the stream. Bulk follows in-queue.
  * Norm: both heads' U-psum evacuations happen before the z/recip/mul math
    (frees the flex PSUM ring early).
  * Epilogue: ALL 16 proj(b1) groups pre-accumulate cc0..4 into a rotating
    PSUM ring and evacuate partials into the y tiles (Scalar engine does the
    copies - it is idle once the last exp is done) while the final norm chain
    drains. After it, only 16 single-matmul cc5 "finals" + in-place adds +
    wide y DMAs on both queues remain.
  * b_proj applied on host (it is a free elementwise add on the output).

Dataflow per core (all-transposed activations; no on-chip transposes):
  host: xT = x_shard^T                                  [C, T]
  qT/kT(hp,b) = Wq/Wk^T-slices @ xT(b)                  [128, N]
  v'   = xT-tiles^T @ Wv  (+ ones col/head)             [N, H*(HD+1)]
  S^T  = k^T-slices^T @ q^T   (per head, K=64)          [Nk, Nq]
  E    = exp(SCALE * S^T)     (ScalarE, PSUM->SBUF)     [Nk, Nq]
  U'   = v'^T @ E  (accum over k; row HD = softmax Z)   [HD+1, Nq]
  aoT  = U'[:HD] * (1/Z broadcast)                      [C, N]
  y    = aoT-tiles^T @ W_proj                           [N, C]
"""

import sys

for _p in ("/opt/trn_rl_repo", "/opt/pypackages"):
    if _p not in sys.path:
        sys.path.append(_p)

import numpy as np

B, N, C, H = 16, 1024, 768, 12
HD = C // H            # 64
SCALE = HD ** -0.5
NCORES = 8
BL = B // NCORES       # 2 batches per core
T = BL * N             # 2048 tokens per core

COMPUTE = "bf16"       # "bf16" | "f32r"


def build_attention_nc(compute=COMPUTE, bl=BL, n=N, c=C, h=H):
    import concourse.bass as bass
    import concourse.tile as tile
    from concourse import bacc, mybir
    from contextlib import ExitStack

    hd = c // h
    t = bl * n
    scale = hd ** -0.5
    assert c % 128 == 0 and n % 512 == 0 and h % 2 == 0 and hd == 64
    CCH = c // 128      # contraction chunks over channels (6)
    NHP = h // 2        # head pairs (6)
    NQ = n // 512       # 512-wide q tiles per sequence (2)
    NKT = n // 128      # 128-wide k tiles per sequence (8)
    NTT = n // 128      # 128-wide token tiles per sequence (8)
    VW = hd + 1         # v' width per head (ones col at hd)
    PH = c // 2         # proj/v free-dim half (384), <= 1 PSUM bank
    NXH = n // 512      # 512-col x halves per batch (2)

    FP32 = mybir.dt.float32
    SD = mybir.dt.bfloat16 if compute == "bf16" else FP32  # storage dtype

    def mm(ap):
        return ap.bitcast(mybir.dt.float32r) if compute == "f32r" else ap

    nc = bacc.Bacc("TRN2", target_bir_lowering=False, debug=False,
                   num_devices=NCORES)

    # host-side PARTITION-MAJOR packed images (see make_in_maps): every DMA
    # is a straight [128, X] contiguous copy at full descriptor efficiency.
    NXG = CCH // 2      # x chunk-pair groups (3)
    xT_d = nc.dram_tensor("xT", [bl * NXG * 128, 2 * n], SD,
                          kind="ExternalInput").ap()
    wq_d = nc.dram_tensor("w_q", [128, NHP * CCH * 128], SD,
                          kind="ExternalInput").ap()      # (hp, cc, f)-major
    wk_d = nc.dram_tensor("w_k", [128, NHP * CCH * 128], SD,
                          kind="ExternalInput").ap()
    wv_d = nc.dram_tensor("w_v", [128, 2 * CCH * PH], SD,
                          kind="ExternalInput").ap()      # (half, cc, f)-major
    wqk0_d = nc.dram_tensor("w_qk0", [128, 2 * CCH * 128], SD,
                            kind="ExternalInput").ap()    # hp0 of wq|wk
    wproj_d = nc.dram_tensor("w_proj", [128, CCH * c], SD,
                             kind="ExternalInput").ap()   # (cc, f)-major
    out_d = nc.dram_tensor("out", [t, c], FP32, kind="ExternalOutput").ap()

    Exp = mybir.ActivationFunctionType.Exp
    Copy = mybir.ActivationFunctionType.Copy

    units = [(b, hp) for b in range(bl) for hp in range(NHP)]
    NU = len(units)     # 12

    with tile.TileContext(nc) as tc, ExitStack() as ctx:
        consts = ctx.enter_context(tc.tile_pool(name="consts", bufs=1))
        xp = ctx.enter_context(tc.tile_pool(name="xp", bufs=2))
        qkp = ctx.enter_context(tc.tile_pool(name="qkp", bufs=2))
        vp = ctx.enter_context(tc.tile_pool(name="vp", bufs=2))
        ep = ctx.enter_context(tc.tile_pool(name="ep", bufs=9))
        aop = ctx.enter_context(tc.tile_pool(name="aop", bufs=2))
        smp = ctx.enter_context(tc.tile_pool(name="smp", bufs=1))
        yp = ctx.enter_context(tc.tile_pool(name="yp", bufs=8))
        ps_s = ctx.enter_context(tc.tile_pool(name="ps_s", bufs=2, space="PSUM"))
        ps_f = ctx.enter_context(tc.tile_pool(name="ps_f", bufs=4, space="PSUM"))

        # ---------------- DMA prologue --------------------------------------
        # All DMAs are straight partition-major copies. Ramp-critical chase:
        #   sync:   x0g0, x0g1, x0g2, wqA(hp1-2), wqB(hp3-5)
        #   gpsimd: wqk0 (hp0 weights, first!), wvA(half0), wvB, wkA, wkB
        #   scalar: x1g0, x1g1, x1g2, wproj
        wqk0_sb = consts.tile([128, 2 * CCH, 128], SD, tag="wqk0")
        wq_hp0 = [wqk0_sb[:, cc, :] for cc in range(CCH)]
        wk_hp0 = [wqk0_sb[:, CCH + cc, :] for cc in range(CCH)]
        xg = [[xp.tile([128, 2, n], SD, tag=f"xg{g}", name=f"xg_b{b}g{g}")
               for g in range(NXG)] for b in range(bl)]
        xT_all = [[[xg[b][cc // 2][:, cc % 2, xh * 512:(xh + 1) * 512]
                    for xh in range(NXH)] for cc in range(CCH)]
                  for b in range(bl)]
        wqt = consts.tile([128, NHP, CCH, 128], SD, tag="wqt")
        wkt = consts.tile([128, NHP, CCH, 128], SD, tag="wkt")
        wvt = consts.tile([128, 2, CCH, PH], SD, tag="wvt")
        wpt = consts.tile([128, CCH, c], SD, tag="wpt")
        wproj_sb = [wpt[:, cc, :] for cc in range(CCH)]

        def wq_ap(cc, hp):
            return wqt[:, hp, cc, :]

        def wk_ap(cc, hp):
            return wkt[:, hp, cc, :]

        def wv_ap(cc, half):
            return wvt[:, half, cc, :]

        def r3(dram_ap, j):
            return dram_ap.rearrange("p (j f) -> p j f", j=j)

        nc.gpsimd.dma_start(out=wqk0_sb, in_=r3(wqk0_d[:, :], 2 * CCH))
        for g in range(NXG):
            nc.sync.dma_start(out=xg[0][g],
                              in_=r3(xT_d[g * 128:(g + 1) * 128, :], 2))
        for half in range(2):
            s0 = half * CCH * PH
            nc.gpsimd.dma_start(
                out=wvt[:, half], in_=r3(wv_d[:, s0:s0 + CCH * PH], CCH))
        for piece, (h0, h1) in enumerate(((1, 3), (3, 6))):
            s0, s1 = h0 * CCH * 128, h1 * CCH * 128
            nc.sync.dma_start(
                out=wqt[:, h0:h1],
                in_=wq_d[:, s0:s1].rearrange("p (hp cc f) -> p hp cc f",
                                             hp=h1 - h0, cc=CCH))
            nc.gpsimd.dma_start(
                out=wkt[:, h0:h1],
                in_=wk_d[:, s0:s1].rearrange("p (hp cc f) -> p hp cc f",
                                             hp=h1 - h0, cc=CCH))
        for g in range(NXG):
            nc.scalar.dma_start(
                out=xg[1][g],
                in_=r3(xT_d[(NXG + g) * 128:(NXG + g + 1) * 128, :], 2))
        nc.scalar.dma_start(out=wpt, in_=r3(wproj_d[:, :], CCH))

        # ---------------- building-block emitters --------------------------
        qt_all = {}   # (b, hp) -> [128, n] q^T tile (2 heads stacked)
        kt_all = {}
        v_all = [[None] * NTT for _ in range(bl)]
        e_all = {}    # (b, hp, kt, head) -> E tile
        u_ps = {}     # (b, hp) -> [head][qn] psum accumulators
        ao_all = {}   # (b, hp) -> [128, n] normalized attention output^T
        y_tiles = {}  # (b, tt) -> ([128, c] tile, halves-finished count)

        def emit_qk_group(b, hp, dst, qn):
            """Project one 512-token slice of q^T (dst=0) or k^T (dst=1)."""
            key = (b, hp)
            store = qt_all if dst == 0 else kt_all
            if key not in store:
                store[key] = qkp.tile([128, n], SD, tag=f"qk{dst}",
                                      name=f"{'qk'[dst]}t_b{b}hp{hp}")
            ps = ps_f.tile([128, 512], FP32, tag="u",
                           name=f"qkps_b{b}hp{hp}d{dst}q{qn}")
            for cc in range(CCH):
                if hp == 0:
                    w_ap = (wq_hp0 if dst == 0 else wk_hp0)[cc]
                else:
                    w_ap = (wq_ap if dst == 0 else wk_ap)(cc, hp)
                nc.tensor.matmul(
                    ps,
                    lhsT=mm(w_ap),
                    rhs=mm(xT_all[b][cc][qn]),
                    start=(cc == 0), stop=(cc == CCH - 1))
            with tc.high_priority(offset=300):
                nc.vector.tensor_copy(
                    store[key][:, qn * 512:(qn + 1) * 512], ps)

        def emit_v_group(b, tt, half):
            """One [128-token, 384-channel] slice of v' (+ones cols)."""
            if half == 0:
                vt = vp.tile([128, h * VW], SD, tag=f"v{tt}",
                             name=f"v_b{b}t{tt}")
                ones_view = vt[:, :].rearrange(
                    "p (hh w) -> p hh w", hh=h)[:, :, hd:hd + 1]
                nc.gpsimd.memset(ones_view, 1.0)
                v_all[b][tt] = vt
            vt = v_all[b][tt]
            ps = ps_f.tile([128, PH], FP32, tag="u",
                           name=f"vps_b{b}t{tt}f{half}")
            xh, tl = tt // 4, tt % 4
            for cc in range(CCH):
                nc.tensor.matmul(
                    ps,
                    lhsT=mm(xT_all[b][cc][xh][:, tl * 128:(tl + 1) * 128]),
                    rhs=mm(wv_ap(cc, half)),
                    start=(cc == 0), stop=(cc == CCH - 1))
            nheads = PH // hd
            dst = vt[:, half * nheads * VW:(half + 1) * nheads * VW].rearrange(
                "p (hh w) -> p hh w", hh=nheads)[:, :, 0:hd]
            srcv = ps[:].rearrange("p (hh w) -> p hh w", hh=nheads)
            with tc.high_priority(offset=300):
                nc.vector.tensor_copy(dst, srcv)

        def get_y_tile(b, tt):
            if (b, tt) not in y_tiles:
                y_tiles[(b, tt)] = [yp.tile([128, c], FP32, tag="y",
                                            name=f"y_b{b}t{tt}"), 0]
            return y_tiles[(b, tt)]

        tail_q = [0]

        def evac_y_half(b, tt, half, ps, evac, bump=True):
            """Copy/accumulate proj psum into the y tile; DMA when complete."""
            ent = get_y_tile(b, tt)
            yt = ent[0]
            dstv = yt[:, half * PH:(half + 1) * PH]
            if evac == "scalar":
                nc.scalar.activation(dstv, ps, Copy)
            elif evac == "add":
                with tc.high_priority(offset=300):
                    nc.vector.tensor_add(dstv, ps, dstv)
            else:
                with tc.high_priority(offset=300):
                    nc.vector.tensor_copy(dstv, ps)
            if bump:
                ent[1] += 1
                if ent[1] == 2:
                    if b == 0:
                        q = nc.sync if tt % 2 == 0 else nc.gpsimd
                    else:
                        qs = [nc.sync, nc.gpsimd, nc.scalar]
                        q = qs[tail_q[0] % 3]
                        tail_q[0] += 1
                    q.dma_start(
                        out=out_d[b * n + tt * 128:b * n + (tt + 1) * 128, :],
                        in_=yt)
                    del y_tiles[(b, tt)]

        def emit_proj_group(b, tt, half, evac="vector"):
            """One [128-token, 384-channel] output-projection slice."""
            ps = ps_f.tile([128, PH], FP32, tag="u",
                           name=f"yps_b{b}t{tt}f{half}")
            for cc in range(CCH):
                nc.tensor.matmul(
                    ps,
                    lhsT=mm(ao_all[(b, cc)][:, tt * 128:(tt + 1) * 128]),
                    rhs=mm(wproj_sb[cc][:, half * PH:(half + 1) * PH]),
                    start=(cc == 0), stop=(cc == CCH - 1))
            evac_y_half(b, tt, half, ps, evac)

        def emit_S(b, hp, kt):
            """S^T matmuls + exp for both heads of one 128-key tile."""
            qb = qt_all[(b, hp)]
            kb = kt_all[(b, hp)]
            for head in range(2):
                p0 = head * 64
                sps = ps_s.tile([128, n], FP32, tag="s",
                                name=f"s_b{b}hp{hp}k{kt}h{head}")
                for qn in range(NQ):
                    nc.tensor.matmul(
                        sps[:, qn * 512:(qn + 1) * 512],
                        lhsT=mm(kb[p0:p0 + 64, kt * 128:(kt + 1) * 128]),
                        rhs=mm(qb[p0:p0 + 64, qn * 512:(qn + 1) * 512]),
                        start=True, stop=True)
                et = ep.tile([128, n], SD, tag=f"e{head}",
                             name=f"e_b{b}hp{hp}k{kt}h{head}")
                nc.scalar.activation(et, sps, Exp, scale=scale)
                e_all[(b, hp, kt, head)] = et

        def emit_U_chunk(b, hp, kts):
            """U matmuls for key-tiles `kts`; their E tiles are all ready."""
            if (b, hp) not in u_ps:
                u_ps[(b, hp)] = [[ps_f.tile([VW, 512], FP32, tag="u",
                                            name=f"u_b{b}hp{hp}h{hh}q{qn}")
                                  for qn in range(NQ)] for hh in range(2)]
            ups = u_ps[(b, hp)]
            for kt in kts:
                for head in range(2):
                    hh = 2 * hp + head
                    et = e_all.pop((b, hp, kt, head))
                    for qn in range(NQ):
                        nc.tensor.matmul(
                            ups[head][qn],
                            lhsT=mm(v_all[b][kt][:, hh * VW:hh * VW + VW]),
                            rhs=mm(et[:, qn * 512:(qn + 1) * 512]),
                            start=(kt == 0), stop=(kt == NKT - 1))

        def emit_norm_evac(b, hp):
            """Copy both heads' U psums to SBUF (frees the flex ring)."""
            ups = u_ps.pop((b, hp))
            usb = {}
            for head in (1, 0):
                ut = smp.tile([VW, n], FP32, tag=f"usb{head}",
                              name=f"usb_b{b}hp{hp}h{head}")
                for qn in range(NQ):
                    with tc.high_priority(offset=300):
                        nc.vector.tensor_copy(
                            ut[:, qn * 512:(qn + 1) * 512], ups[head][qn])
                usb[head] = ut
            return usb

        def emit_norm_math(b, hp, usb):
            """Divide by Z, build aoT tile for proj.

            Z -> partition 0 (DMA), broadcast to 64 partitions (gpsimd),
            reciprocal on the full tile (base-partition!=0 sources
            mis-execute on hw for both the DVE op and the broadcast).
            """
            ao = aop.tile([128, n], SD, tag=f"ao{hp}", name=f"ao_b{b}hp{hp}")
            ao_all[(b, hp)] = ao
            for head in (1, 0):
                ut = usb[head]
                z1 = smp.tile([1, n], FP32, tag=f"z1{head}", bufs=1,
                              name=f"z1_b{b}hp{hp}h{head}")
                nc.sync.dma_start(out=z1, in_=ut[hd:hd + 1, :])
                rb = smp.tile([64, n], FP32, tag=f"rb{head}",
                              name=f"rb_b{b}hp{hp}h{head}")
                nc.gpsimd.partition_broadcast(rb, z1)
                nc.vector.reciprocal_approx_fast(rb, rb)
                if head == 0:
                    nc.vector.tensor_mul(ao[0:64, :], ut[0:hd, :], rb)
                else:
                    sc = smp.tile([64, n], SD, tag="sc",
                                  name=f"sc_b{b}hp{hp}")
                    nc.vector.tensor_mul(sc, ut[0:hd, :], rb)
                    nc.sync.dma_start(out=ao[64:128, :], in_=sc)

        def emit_norm(b, hp):
            emit_norm_math(b, hp, emit_norm_evac(b, hp))

        # ---------------- filler schedule ----------------------------------
        # per-unit list of thunks run between S groups of that unit
        fillers = [[] for _ in range(NU)]

        def add_qk_fillers(i, b, hp):
            for qn in range(NQ):
                for dst in range(2):
                    fillers[i].append(
                        lambda b=b, hp=hp, dst=dst, qn=qn:
                        emit_qk_group(b, hp, dst, qn))

        # unit 0: v(b0) fully (half0 first: its wv piece lands first) + qk(u1)
        for half in range(2):
            for tt in range(NTT):
                fillers[0].append(
                    lambda tt=tt, half=half: emit_v_group(0, tt, half))
        add_qk_fillers(0, *units[1])
        # units 1..4: qk(next) + v(b1) spread 4 per unit
        for i in range(1, 5):
            add_qk_fillers(i, *units[i + 1])
        vq = [(tt, half) for tt in range(NTT) for half in range(2)]
        for j, (tt, half) in enumerate(vq):
            fillers[1 + j // 4].append(
                lambda tt=tt, half=half: emit_v_group(1, tt, half))
        # units 5..10: qk(next)
        for i in range(5, 11):
            add_qk_fillers(i, *units[i + 1])
        # units 7..10: proj(b0)  (all ao(b0) ready after norm(u5) in unit 6;
        # unit 11 keeps its flex psum free for in-unit U accumulation)
        pq = [(tt, half) for tt in range(NTT) for half in range(2)]
        for j, (tt, half) in enumerate(pq):
            fillers[7 + j % 4].append(
                lambda tt=tt, half=half: emit_proj_group(0, tt, half))

        # ---------------- main schedule ------------------------------------
        # prologue: qk(unit0), qn-major to chase the x DMA stream
        b0, hp0 = units[0]
        for qn in range(NQ):
            for dst in range(2):
                emit_qk_group(b0, hp0, dst, qn)

        for i, (b, hp) in enumerate(units):
            fl = list(fillers[i])
            prev = units[i - 1] if i > 0 else None
            # S(k0) first so the Scalar engine stays fed across the boundary;
            # previous unit's U matmuls run in chunks between S groups so
            # exp never starves and every U operand is long since ready.
            emit_S(b, hp, 0)
            start_kt = 1
            if prev is not None:
                for j, (k0, k1) in enumerate(((0, 2), (2, 4), (4, 6), (6, 8))):
                    emit_U_chunk(prev[0], prev[1], range(k0, k1))
                    if j < 3:
                        emit_S(b, hp, j + 1)
                emit_norm(*prev)
                start_kt = 4
            # spread fillers across the remaining kt slots; the last unit
            # instead runs its own U matmuls in-unit at lag 4
            nslots = NKT - start_kt
            tot = len(fl)
            for kt in range(start_kt, NKT):
                emit_S(b, hp, kt)
                if i == NU - 1 and kt >= 4:
                    emit_U_chunk(b, hp, [kt - 4])
                j = kt - start_kt
                k = (tot * (j + 1)) // nslots - (tot * j) // nslots
                for _ in range(k):
                    if fl:
                        fl.pop(0)()

        # ---------------- epilogue -----------------------------------------
        # Last unit's U(k4..7); 4 warm groups (tt0-1) keep their cc0..4
        # partials IN the freed S-slot banks (final = one more accumulating
        # matmul + plain copy evac), the other 12 groups' partials rotate
        # the flex ring and land in the y tiles (in-place add finals), all
        # while the final norm chain drains. Whole-row y DMAs on 3 queues.
        pb, php = units[-1]
        emit_U_chunk(pb, php, range(4, NKT))

        warm = [(0, 0), (0, 1), (1, 0), (1, 1)]
        rest = [(tt, half) for tt in range(2, NTT) for half in range(2)]
        s_carve = [ps_s.tile([128, n], FP32, tag="s", name=f"scarve{j}")
                   for j in range(2)]
        warm_ps = {}

        def proj_mms(tt, half, ps, ccs, start, stop):
            for cc in ccs:
                nc.tensor.matmul(
                    ps,
                    lhsT=mm(ao_all[(1, cc)][:, tt * 128:(tt + 1) * 128]),
                    rhs=mm(wproj_sb[cc][:, half * PH:(half + 1) * PH]),
                    start=start and cc == ccs[0],
                    stop=stop and cc == ccs[-1])

        for g, (tt, half) in enumerate(warm):
            ps = s_carve[g // 2][:, (g % 2) * 512:(g % 2) * 512 + PH]
            warm_ps[(tt, half)] = ps
            proj_mms(tt, half, ps, range(CCH - 1), True, False)
        usb_last = emit_norm_evac(pb, php)
        for g, (tt, half) in enumerate(rest):
            ps = ps_f.tile([128, PH], FP32, tag="u", name=f"ypart{g}")
            proj_mms(tt, half, ps, range(CCH - 1), True, True)
            evac_y_half(1, tt, half, ps,
                        "scalar" if g % 2 == 0 else "vector", bump=False)
        emit_norm_math(pb, php, usb_last)
        # finals, tt-major so whole-row DMAs start as early as possible
        cc5 = [CCH - 1]
        for tt in range(NTT):
            for half in range(2):
                if (tt, half) in warm_ps:
                    ps = warm_ps[(tt, half)]
                    proj_mms(tt, half, ps, cc5, False, True)
                    evac_y_half(1, tt, half, ps, "scalar")
                else:
                    ps = ps_f.tile([128, PH], FP32, tag="u",
                                   name=f"yfin{tt}_{half}")
                    proj_mms(tt, half, ps, cc5, True, True)
                    evac_y_half(1, tt, half, ps, "add")

    nc.compile()
    return nc


_NC_CACHE = {}


def _get_nc(compute=COMPUTE):
    if compute not in _NC_CACHE:
        _NC_CACHE[compute] = build_attention_nc(compute)
    return _NC_CACHE[compute]


def make_in_maps(x, W_qkv, W_proj, b_proj, compute=None):
    compute = compute or COMPUTE
    if compute == "bf16":
        import ml_dtypes
        sd = ml_dtypes.bfloat16
    else:
        sd = np.float32
    x = np.asarray(x, dtype=np.float32)
    W_qkv = np.asarray(W_qkv, dtype=np.float32)
    CCH, NHP, PH, NXG = C // 128, H // 2, C // 2, C // 256

    def pack(w, inner):
        # [C, X] -> partition-major [128, (outer..., inner)] image
        return np.ascontiguousarray(
            w.reshape(CCH, 128, -1, inner).transpose(1, 2, 0, 3)
            .reshape(128, -1)).astype(sd)

    wq = pack(W_qkv[:, 0:C], 128)              # (hp, cc, 128)
    wk = pack(W_qkv[:, C:2 * C], 128)
    wv = pack(W_qkv[:, 2 * C:3 * C], PH)       # (half, cc, PH)
    wp = pack(np.asarray(W_proj, dtype=np.float32), C)  # (cc, C)
    q0 = W_qkv[:, 0:128].reshape(CCH, 128, 128).transpose(1, 0, 2)
    k0 = W_qkv[:, C:C + 128].reshape(CCH, 128, 128).transpose(1, 0, 2)
    wqk0 = np.ascontiguousarray(
        np.concatenate([q0, k0], axis=1).reshape(128, -1)).astype(sd)
    in_maps = []
    for i in range(NCORES):
        shard = x[i * BL:(i + 1) * BL]                      # [BL, N, C]
        # (b, g, p, j, f) image: rows (b, g, p), cols (j, f)
        xT = np.ascontiguousarray(
            shard.transpose(0, 2, 1).reshape(BL, NXG, 2, 128, N)
            .transpose(0, 1, 3, 2, 4).reshape(BL * NXG * 128, 2 * N)
        ).astype(sd)
        in_maps.append({"xT": xT, "w_q": wq, "w_k": wk, "w_v": wv,
                        "w_qk0": wqk0, "w_proj": wp})
    return in_maps


def kernel(x, W_qkv, W_proj, b_proj):
    from concourse.bass_utils import run_bass_kernel_spmd

    nc = _get_nc()
    in_maps = make_in_maps(x, W_qkv, W_proj, b_proj)
    res = run_bass_kernel_spmd(nc, in_maps, core_ids=list(range(NCORES)))
    outs = [res.results[i]["out"].reshape(BL, N, C) for i in range(NCORES)]
    y = np.concatenate(outs, axis=0).astype(np.float32)
    return y + np.asarray(b_proj, dtype=np.float32)


if __name__ == "__main__":
    nc = build_attention_nc()
    print("built ok")
